# revision 13
# baseline (speedup 1.0000x reference)
"""Trainium2 Bass kernel: causal MHSA, last-position output (bf16 datapath).

The reference returns only out[:, -1, :]; with the causal mask the last query
row attends to everything, so per batch element the whole MHSA collapses to
tiny GEMVs.  q_row and M = Wk-contracted-with-q fold on the host (as in the
fp32 baseline); everything that streams through the device is cast to bf16 on
the host, which halves HBM traffic and runs the PE at 1 cyc/row instead of 4.

Sharding: pure data parallel over batch, core b <- batch b, no collectives.

Device-side structure per core:
  - 5 HWDGE DMAs total: x in 4 chunks (chunk 0 also carries the folded M and
    bo packed into extra columns) and one packed Wv|Wo tensor.  Fewer, larger
    DMAs matter because descriptor-gen serializes at ~650ns per DMA on the
    shared HWDGE device.
  - per chunk: PE transposes x tiles (bf16, 1cyc/row) -> PSUM, copy back to
    SBUF (DVE), scores matmuls (xT stationary, M moving, N=8), ACT exp into
    bf16 wt, attention matmuls (x stationary, wt moving, N=8) accumulating
    axc in PSUM, and softmax sums accumulated in the SAME PSUM tile in the
    block-diag [128,4] layout via 64-row tile_position matmuls (even/odd
    head columns), which makes the final normalize a plain elementwise mul.
  - tail: reciprocal of the sums block, axc -> bf16, per-head 64-row Wv
    matmuls directly into the extracted [128,4] layout (tile_position),
    normalize, Wo matmuls, bias add, store via SWDGE prepare/trigger
    (descriptor-gen off the critical path; saves ~1.2us vs a plain HWDGE
    store issued after the data is ready).
"""

import numpy as np
from contextlib import ExitStack

import concourse.bass as bass
import concourse.tile as tile
from concourse import bacc, mybir
from concourse.bass_utils import run_bass_kernel_spmd
from concourse.masks import make_identity

B, S, F, PROJ, H, D = 8, 2048, 256, 512, 8, 64
NT = S // 128         # 16 s-tiles
FC = F // 128         # 2 f-chunks
CHUNKS = (6, 5, 4, 1)  # tiles per x DMA
NCH = len(CHUNKS)
AUX = 18              # extra cols on chunk0: M (16) + bo (2)
f32 = mybir.dt.float32
bf16 = mybir.dt.bfloat16
i32 = mybir.dt.int32
EXP = mybir.ActivationFunctionType.Exp
COPY = mybir.ActivationFunctionType.Copy

N_WARM = 20           # PE warm-up transposes (p-state ramp)
USE_KV_STORE = True   # SWDGE prepare/trigger store vs plain HWDGE store
COPY_ENG = ("pool", "dve", "dve", "dve")  # xT copy engine per chunk
W_SPLIT = True        # load Wv and Wo with separate DMAs

_cache = {}

_CH_OFF = [sum(CHUNKS[:g]) for g in range(NCH)]  # first tile of each chunk


def _build():
    nc = bacc.Bacc("TRN2", target_bir_lowering=False, debug=False, num_devices=B)
    x_dram = []
    for g, ct in enumerate(CHUNKS):
        cols = ct * F + (AUX if g == 0 else 0)
        x_dram.append(nc.dram_tensor(f"x{g}", [128, cols], bf16, kind="ExternalInput").ap())
    w_dram = nc.dram_tensor("W", [128, 2048], bf16, kind="ExternalInput").ap()
    if USE_KV_STORE:
        out = nc.dram_tensor("out", [1, 128, FC, 1], f32, kind="ExternalOutput").ap()
    else:
        out = nc.dram_tensor("out", [128, FC], f32, kind="ExternalOutput").ap()

    with tile.TileContext(nc) as tc, ExitStack() as ctx:
        P = ctx.enter_context(tc.tile_pool(name="persist", bufs=1))
        xtp = ctx.enter_context(tc.tile_pool(name="xtp", bufs=2, space="PSUM"))
        sct = ctx.enter_context(tc.tile_pool(name="sct", bufs=2, space="PSUM"))
        pers = ctx.enter_context(tc.tile_pool(name="pers", bufs=1, space="PSUM"))
        warmp = ctx.enter_context(tc.tile_pool(name="warmp", bufs=1, space="PSUM"))

        ident = P.tile([128, 128], bf16)
        ones64 = P.tile([128, 64], bf16)
        scratch = P.tile([128, 128], bf16)
        x_sb = [P.tile([128, ct * F + (AUX if g == 0 else 0)], bf16, name=f"x_sb{g}")
                for g, ct in enumerate(CHUNKS)]
        xT_sb = P.tile([128, FC, S], bf16)
        wt_sb = P.tile([128, NT * H], bf16)
        axT_sb = P.tile([128, FC * H], bf16)
        bd_sb = P.tile([128, 4], f32)
        ac_sb = P.tile([128, 4], bf16)
        o_sb = P.tile([128, FC], f32)
        w_sb = P.tile([128, 2048], bf16)
        if USE_KV_STORE:
            idx_sb = P.tile([128, 1], i32)

        def xv(t, c):
            """x tile t, f-chunk c as a [128,128] SBUF view ([s-part, f])."""
            g = 0
            while g + 1 < NCH and t >= _CH_OFF[g + 1]:
                g += 1
            j = t - _CH_OFF[g]
            base = j * F + c * 128
            return x_sb[g][:, base : base + 128]

        def mv(c):
            return x_sb[0][:, CHUNKS[0] * F + c * H : CHUNKS[0] * F + (c + 1) * H]

        bo_v = x_sb[0][:, CHUNKS[0] * F + 2 * H : CHUNKS[0] * F + 2 * H + FC]

        # ---- init (gpsimd builds constants; DVE memsets warm-up scratch)
        nc.vector.memset(scratch[:], 1.0)
        make_identity(nc, ident[:])
        nc.gpsimd.memset(ones64[:], 1.0)
        if USE_KV_STORE:
            nc.gpsimd.memset(idx_sb[:], 0)

        # ---- DMAs (SP/HWDGE): x chunks then weights
        for g in range(NCH):
            nc.sync.dma_start(out=x_sb[g][:], in_=x_dram[g])
        if W_SPLIT:
            nc.sync.dma_start(out=w_sb[:, 0:1024], in_=w_dram[:, 0:1024])
            nc.sync.dma_start(out=w_sb[:, 1024:2048], in_=w_dram[:, 1024:2048])
        else:
            nc.sync.dma_start(out=w_sb[:], in_=w_dram)

        def wv_v(fc, c, half):
            return w_sb[:, fc * 512 + c * 128 + 64 * half : fc * 512 + c * 128 + 64 * half + 64]

        def wo_v(c, mc):
            return w_sb[:, 1024 + c * 256 + mc * 128 : 1024 + c * 256 + (mc + 1) * 128]

        # ---- PE warm-up (p-state ramp; content is irrelevant)
        warm_ps = warmp.tile([128, 128], bf16, tag="warm")
        for _ in range(N_WARM):
            nc.tensor.transpose(warm_ps[:], scratch[:], scratch[:])

        # persistent PSUM accumulators: axc in cols 0..16, sums in 16..20
        axs_ps = pers.tile([128, FC * H + 4], f32, tag="axs")

        def emit_transposes(g):
            xt_ps = xtp.tile([128, FC, CHUNKS[g] * 128], bf16, tag="xt", name=f"xt{g}")
            for j in range(CHUNKS[g]):
                t = _CH_OFF[g] + j
                for c in range(FC):
                    nc.tensor.transpose(
                        xt_ps[:, c, j * 128 : (j + 1) * 128], xv(t, c), ident[:]
                    )
            return xt_ps

        def emit_copy(g, xt_ps):
            lo = _CH_OFF[g] * 128
            dst = xT_sb[:, :, lo : lo + CHUNKS[g] * 128]
            eng = COPY_ENG[g]
            if eng == "dve":
                nc.vector.tensor_copy(dst, xt_ps[:])
            elif eng == "pool":
                nc.gpsimd.tensor_copy(dst, xt_ps[:])
            elif eng == "act":
                nc.scalar.activation(out=dst, in_=xt_ps[:], func=COPY)
            else:
                raise ValueError(eng)

        def emit_scores(g):
            sct_ps = sct.tile([128, CHUNKS[g] * H], f32, tag="sc", name=f"sc{g}")
            for j in range(CHUNKS[g]):
                t = _CH_OFF[g] + j
                for c in range(FC):
                    nc.tensor.matmul(
                        sct_ps[:, j * H : (j + 1) * H],
                        xT_sb[:, c, t * 128 : (t + 1) * 128],
                        mv(c),
                        start=(c == 0),
                        stop=(c == FC - 1),
                    )
            return sct_ps

        def emit_exp(g, sct_ps):
            nc.scalar.activation(
                out=wt_sb[:, _CH_OFF[g] * H : (_CH_OFF[g] + CHUNKS[g]) * H],
                in_=sct_ps[:],
                func=EXP,
                scale=0.125,
            )

        def emit_attn(g):
            for j in range(CHUNKS[g]):
                t = _CH_OFF[g] + j
                wtt = wt_sb[:, t * H : (t + 1) * H]
                for c in range(FC):
                    nc.tensor.matmul(
                        axs_ps[:, c * H : (c + 1) * H],
                        xv(t, c),
                        wtt,
                        start=(t == 0),
                        stop=(t == NT - 1),
                        skip_group_check=True,
                    )
                # softmax sums accumulated directly in the block-diag [128,4]
                # layout: rows 0:64 <- even heads, rows 64:128 <- odd heads
                for half in range(2):
                    sel = bass.AP(
                        tensor=wtt.tensor, offset=wtt.offset + half, ap=[wtt.ap[0], [2, 4]]
                    )
                    nc.tensor.matmul(
                        axs_ps[64 * half : 64 * (half + 1), FC * H : FC * H + 4],
                        ones64[:],
                        sel,
                        start=(t == 0),
                        stop=(t == NT - 1),
                        skip_group_check=True,
                        tile_position=(0, 64 * half),
                    )

        # ---- software-pipelined emission (per-engine queues are in-order)
        xt0 = emit_transposes(0)
        xt1 = emit_transposes(1)
        emit_copy(0, xt0)
        sc0 = emit_scores(0)
        xt2 = emit_transposes(2)
        emit_copy(1, xt1)
        emit_exp(0, sc0)
        sc1 = emit_scores(1)
        emit_attn(0)
        xt3 = emit_transposes(3)
        emit_copy(2, xt2)
        emit_exp(1, sc1)
        sc2 = emit_scores(2)
        emit_attn(1)
        emit_copy(3, xt3)
        emit_exp(2, sc2)
        sc3 = emit_scores(3)
        emit_attn(2)
        emit_exp(3, sc3)
        emit_attn(3)

        # ---- tail: recip, axc->bf16, per-head Wv matmuls straight into the
        #      extracted [128,4] layout, normalize, Wo matmuls, bias, store
        nc.vector.tensor_copy(axT_sb[:], axs_ps[:, 0 : FC * H])
        nc.vector.reciprocal(bd_sb[:], axs_ps[:, FC * H : FC * H + 4])

        afT_ps = warmp.tile([128, 4], f32, tag="warm", name="afT")
        for c in range(4):
            for half in range(2):
                head = 2 * c + half
                for fc in range(FC):
                    nc.tensor.matmul(
                        afT_ps[64 * half : 64 * (half + 1), c : c + 1],
                        wv_v(fc, c, half),
                        axT_sb[:, fc * H + head : fc * H + head + 1],
                        start=(fc == 0),
                        stop=(fc == FC - 1),
                        skip_group_check=True,
                        tile_position=(0, 64 * half),
                    )
        nc.vector.tensor_mul(ac_sb[:], afT_ps[:], bd_sb[:])

        o_ps = warmp.tile([128, FC], f32, tag="warm", name="o")
        for mc in range(FC):
            for c in range(4):
                nc.tensor.matmul(
                    o_ps[:, mc : mc + 1],
                    wo_v(c, mc),
                    ac_sb[:, c : c + 1],
                    start=(c == 0),
                    stop=(c == 3),
                    skip_group_check=True,
                )
        nc.vector.tensor_add(o_sb[:], o_ps[:], bo_v)

        if USE_KV_STORE:
            # prep emitted AFTER the o_sb writer so the deferred RAW edge
            # lands on the trigger (the prep itself runs early on the idle
            # Pool engine — it has no sync deps of its own)
            kv_sem = nc.alloc_semaphore("kv_done")
            nc.gpsimd.kv_writeback(
                out_ap=out,
                in_ap=o_sb[:].rearrange("p (a b c) -> p a b c", b=1, c=1),
                ctx_idxs_ap=idx_sb[:],
                prepare_only=True,
                sem=kv_sem,
            )
            nc.gpsimd.trigger_dma(count=None)
            nc.gpsimd.wait_ge(kv_sem, 16)
        else:
            nc.sync.dma_start(out=out, in_=o_sb[:])

    if USE_KV_STORE:
        # Tile's pass 1 gives the SWDGE prep a DMASW lane tick, so the exit
        # drain waits DMASW0 >= 16 — but with a custom sem= the increments
        # went to kv_done instead.  Append the DMASW bump to the prep's
        # updates (fires at prep completion; program end is still gated on
        # the real DMA via wait_ge(kv_done)).
        insts = [i for blk in nc.m.functions[0].blocks for i in blk.instructions]
        dmasw = None
        for i in insts:
            si = i.sync_info
            if not si:
                continue
            for w in si.on_wait:
                if (w.ant_name or "").startswith("DMASW"):
                    dmasw = w
                    break
            if dmasw:
                break
        assert dmasw is not None, "no DMASW drain wait found"
        prep_idx = next(
            k for k, i in enumerate(insts) if type(i).__name__ == "InstKVWritebackAnt"
        )
        prep = insts[prep_idx]
        upd = mybir.SyncUpdate(
            sync_type="semaphore",
            id=dmasw.id,
            ant_name=dmasw.ant_name,
            update_mode="sem-add-imm",
            update_value=dmasw.wait_value,
            update_reg=None,
        )
        si = prep.sync_info
        si.on_update = list(si.on_update) + [upd]
        # The kv prep's o_sb RAW edge is NOT in the rust swdge deferred-deps
        # table, so tile emitted a Pool EventSemaphore (wait DVE tick of the
        # o_sb writer) right before the prep — putting the ~1us descriptor
        # gen on the critical path.  Desc-gen reads no tensor data, so move
        # that wait onto the trigger instead (which is when the DMA actually
        # reads o_sb).
        trig = next(i for i in insts if type(i).__name__ == "InstTriggerDma")
        moved = []
        psi = prep.sync_info
        keep = []
        for w in psi.on_wait:
            (keep if w.ant_name.startswith("Pool") else moved).append(w)
        psi.on_wait = keep
        for k in range(prep_idx - 1, max(prep_idx - 4, -1), -1):
            ii = insts[k]
            if type(ii).__name__ == "InstEventSemaphore" and ii.sync_info:
                wsi = ii.sync_info
                ev_keep = []
                for w in wsi.on_wait:
                    (ev_keep if w.ant_name.startswith("Pool") else moved).append(w)
                wsi.on_wait = ev_keep
                break
        assert moved, "no data wait found to defer from kv prep to trigger"
        tsi = trig.sync_info
        tsi.on_wait = list(tsi.on_wait) + moved

    nc.compile()
    return nc


def get_nc():
    if "nc" not in _cache:
        _cache["nc"] = _build()
    return _cache["nc"]


def host_prep(inputs: dict) -> list[dict]:
    """Per-core input maps: bf16 x chunks (chunk0 carries folded M and bo)
    plus a shared packed bf16 Wv|Wo tensor."""
    import ml_dtypes

    bf = ml_dtypes.bfloat16
    xs = np.asarray(inputs["x"], dtype=np.float32)
    Wq = np.asarray(inputs["Wq"], dtype=np.float32)
    Wk = np.asarray(inputs["Wk"], dtype=np.float32)
    Wv = np.asarray(inputs["Wv"], dtype=np.float32)
    Wo = np.asarray(inputs["Wo"], dtype=np.float32)
    bo = np.asarray(inputs["bo"], dtype=np.float32)

    w_pack = np.concatenate(
        [
            Wv.reshape(FC, 128, PROJ).transpose(1, 0, 2).reshape(128, FC * PROJ),
            Wo.reshape(4, 128, F).transpose(1, 0, 2).reshape(128, 4 * F),
        ],
        axis=1,
    ).astype(bf)
    bo_pack = np.ascontiguousarray(bo.reshape(FC, 128).T)

    in_maps = []
    for b in range(B):
        q_row = xs[b, -1] @ Wq                                   # [512]
        M = (Wk * q_row[None, :]).reshape(F, H, D).sum(-1)       # [256, 8]
        m_pack = M.reshape(FC, 128, H).transpose(1, 0, 2).reshape(128, FC * H)
        flat = xs[b].reshape(NT, 128, F).transpose(1, 0, 2).reshape(128, NT * F)
        m = {"W": w_pack}
        for g, ct in enumerate(CHUNKS):
            lo = _CH_OFF[g] * F
            part = flat[:, lo : lo + ct * F]
            if g == 0:
                part = np.concatenate([part, m_pack, bo_pack], axis=1)
            m[f"x{g}"] = np.ascontiguousarray(part.astype(bf))
        in_maps.append(m)
    return in_maps


def run_hw(inputs: dict) -> np.ndarray:
    nc = get_nc()
    res = run_bass_kernel_spmd(nc, host_prep(inputs), list(range(B)))
    outs = []
    for b in range(B):
        o = np.asarray(res.results[b]["out"], dtype=np.float32).reshape(128, FC)
        outs.append(o.T.reshape(F))
    return np.stack(outs)


def kernel(**inputs) -> np.ndarray:
    return run_hw(inputs)


# revision 14
# speedup vs baseline: 1.0456x; 1.0456x over previous
"""Trainium2 Bass kernel: causal MHSA, last-position output (bf16 datapath).

The reference returns only out[:, -1, :]; with the causal mask the last query
row attends to everything, so per batch element the whole MHSA collapses to
tiny GEMVs.  q_row and M = Wk-contracted-with-q fold on the host (as in the
fp32 baseline); everything that streams through the device is cast to bf16 on
the host, which halves HBM traffic and runs the PE at 1 cyc/row instead of 4.

Sharding: pure data parallel over batch, core b <- batch b, no collectives.

Device-side structure per core:
  - 5 HWDGE DMAs total: x in 4 chunks (chunk 0 also carries the folded M and
    bo packed into extra columns) and one packed Wv|Wo tensor.  Fewer, larger
    DMAs matter because descriptor-gen serializes at ~650ns per DMA on the
    shared HWDGE device.
  - per chunk: PE transposes x tiles (bf16, 1cyc/row) -> PSUM, copy back to
    SBUF (DVE), scores matmuls (xT stationary, M moving, N=8), ACT exp into
    bf16 wt, attention matmuls (x stationary, wt moving, N=8) accumulating
    axc in PSUM, and softmax sums accumulated in the SAME PSUM tile in the
    block-diag [128,4] layout via 64-row tile_position matmuls (even/odd
    head columns), which makes the final normalize a plain elementwise mul.
  - tail: reciprocal of the sums block, axc -> bf16, per-head 64-row Wv
    matmuls directly into the extracted [128,4] layout (tile_position),
    normalize, Wo matmuls, bias add, store via SWDGE prepare/trigger
    (descriptor-gen off the critical path; saves ~1.2us vs a plain HWDGE
    store issued after the data is ready).
"""

import numpy as np
from contextlib import ExitStack

import concourse.bass as bass
import concourse.tile as tile
from concourse import bacc, mybir
from concourse.bass_utils import run_bass_kernel_spmd
from concourse.masks import make_identity

B, S, F, PROJ, H, D = 8, 2048, 256, 512, 8, 64
NT = S // 128         # 16 s-tiles
FC = F // 128         # 2 f-chunks
import os as _os

CHUNKS = tuple(int(v) for v in _os.environ.get("K_CHUNKS", "5,5,4,2").split(","))
NCH = len(CHUNKS)
AUX = 18              # extra cols on chunk0: M (16) + bo (2)
f32 = mybir.dt.float32
bf16 = mybir.dt.bfloat16
i32 = mybir.dt.int32
EXP = mybir.ActivationFunctionType.Exp
COPY = mybir.ActivationFunctionType.Copy

N_WARM = int(_os.environ.get("K_WARM", "20"))
USE_KV_STORE = _os.environ.get("K_KV", "1") == "1"
COPY_ENG = tuple(_os.environ.get("K_COPY", "dve,dve,dve,dve").split(","))
W_SPLIT = _os.environ.get("K_WSPLIT", "1") == "1"

_cache = {}

_CH_OFF = [sum(CHUNKS[:g]) for g in range(NCH)]  # first tile of each chunk


def _build():
    nc = bacc.Bacc("TRN2", target_bir_lowering=False, debug=False, num_devices=B)
    x_dram = []
    for g, ct in enumerate(CHUNKS):
        cols = ct * F + (AUX if g == 0 else 0)
        x_dram.append(nc.dram_tensor(f"x{g}", [128, cols], bf16, kind="ExternalInput").ap())
    w_dram = nc.dram_tensor("W", [128, 2048], bf16, kind="ExternalInput").ap()
    if USE_KV_STORE:
        out = nc.dram_tensor("out", [1, 128, FC, 1], f32, kind="ExternalOutput").ap()
    else:
        out = nc.dram_tensor("out", [128, FC], f32, kind="ExternalOutput").ap()

    with tile.TileContext(nc) as tc, ExitStack() as ctx:
        P = ctx.enter_context(tc.tile_pool(name="persist", bufs=1))
        xtp = ctx.enter_context(tc.tile_pool(name="xtp", bufs=2, space="PSUM"))
        sct = ctx.enter_context(tc.tile_pool(name="sct", bufs=2, space="PSUM"))
        pers = ctx.enter_context(tc.tile_pool(name="pers", bufs=1, space="PSUM"))
        warmp = ctx.enter_context(tc.tile_pool(name="warmp", bufs=1, space="PSUM"))

        ident = P.tile([128, 128], bf16)
        ones64 = P.tile([128, 64], bf16)
        scratch = P.tile([128, 128], bf16)
        x_sb = [P.tile([128, ct * F + (AUX if g == 0 else 0)], bf16, name=f"x_sb{g}")
                for g, ct in enumerate(CHUNKS)]
        xT_sb = P.tile([128, FC, S], bf16)
        wt_sb = P.tile([128, NT * H], bf16)
        axT_sb = P.tile([128, FC * H], bf16)
        bd_sb = P.tile([128, 4], f32)
        ac_sb = P.tile([128, 4], bf16)
        o_sb = P.tile([128, FC], f32)
        w_sb = P.tile([128, 2048], bf16)
        if USE_KV_STORE:
            idx_sb = P.tile([128, 1], i32)

        def xv(t, c):
            """x tile t, f-chunk c as a [128,128] SBUF view ([s-part, f])."""
            g = 0
            while g + 1 < NCH and t >= _CH_OFF[g + 1]:
                g += 1
            j = t - _CH_OFF[g]
            base = j * F + c * 128
            return x_sb[g][:, base : base + 128]

        def mv(c):
            return x_sb[0][:, CHUNKS[0] * F + c * H : CHUNKS[0] * F + (c + 1) * H]

        bo_v = x_sb[0][:, CHUNKS[0] * F + 2 * H : CHUNKS[0] * F + 2 * H + FC]

        # ---- init (gpsimd builds constants; DVE memsets warm-up scratch)
        nc.vector.memset(scratch[:], 1.0)
        make_identity(nc, ident[:])
        nc.gpsimd.memset(ones64[:], 1.0)
        if USE_KV_STORE:
            nc.gpsimd.memset(idx_sb[:], 0)

        # ---- DMAs (SP/HWDGE): x chunks then weights
        for g in range(NCH):
            nc.sync.dma_start(out=x_sb[g][:], in_=x_dram[g])
        if W_SPLIT:
            nc.sync.dma_start(out=w_sb[:, 0:1024], in_=w_dram[:, 0:1024])
            nc.sync.dma_start(out=w_sb[:, 1024:2048], in_=w_dram[:, 1024:2048])
        else:
            nc.sync.dma_start(out=w_sb[:], in_=w_dram)

        def wv_v(fc, c, half):
            return w_sb[:, fc * 512 + c * 128 + 64 * half : fc * 512 + c * 128 + 64 * half + 64]

        def wo_v(c, mc):
            return w_sb[:, 1024 + c * 256 + mc * 128 : 1024 + c * 256 + (mc + 1) * 128]

        # ---- PE warm-up (p-state ramp; content is irrelevant)
        warm_ps = warmp.tile([128, 128], bf16, tag="warm")
        for _ in range(N_WARM):
            nc.tensor.transpose(warm_ps[:], scratch[:], scratch[:])

        # persistent PSUM accumulators: axc in cols 0..16, sums in 16..20
        axs_ps = pers.tile([128, FC * H + 4], f32, tag="axs")

        def emit_transposes(g):
            xt_ps = xtp.tile([128, FC, CHUNKS[g] * 128], bf16, tag="xt", name=f"xt{g}")
            for j in range(CHUNKS[g]):
                t = _CH_OFF[g] + j
                for c in range(FC):
                    nc.tensor.transpose(
                        xt_ps[:, c, j * 128 : (j + 1) * 128], xv(t, c), ident[:]
                    )
            return xt_ps

        def emit_copy(g, xt_ps):
            lo = _CH_OFF[g] * 128
            dst = xT_sb[:, :, lo : lo + CHUNKS[g] * 128]
            eng = COPY_ENG[g]
            if eng == "dve":
                nc.vector.tensor_copy(dst, xt_ps[:])
            elif eng == "pool":
                nc.gpsimd.tensor_copy(dst, xt_ps[:])
            elif eng == "act":
                nc.scalar.activation(out=dst, in_=xt_ps[:], func=COPY)
            else:
                raise ValueError(eng)

        def emit_scores(g):
            sct_ps = sct.tile([128, CHUNKS[g] * H], f32, tag="sc", name=f"sc{g}")
            for j in range(CHUNKS[g]):
                t = _CH_OFF[g] + j
                for c in range(FC):
                    nc.tensor.matmul(
                        sct_ps[:, j * H : (j + 1) * H],
                        xT_sb[:, c, t * 128 : (t + 1) * 128],
                        mv(c),
                        start=(c == 0),
                        stop=(c == FC - 1),
                    )
            return sct_ps

        def emit_exp(g, sct_ps):
            nc.scalar.activation(
                out=wt_sb[:, _CH_OFF[g] * H : (_CH_OFF[g] + CHUNKS[g]) * H],
                in_=sct_ps[:],
                func=EXP,
                scale=0.125,
            )

        def emit_attn(g):
            for j in range(CHUNKS[g]):
                t = _CH_OFF[g] + j
                wtt = wt_sb[:, t * H : (t + 1) * H]
                for c in range(FC):
                    nc.tensor.matmul(
                        axs_ps[:, c * H : (c + 1) * H],
                        xv(t, c),
                        wtt,
                        start=(t == 0),
                        stop=(t == NT - 1),
                        skip_group_check=True,
                    )
                # softmax sums accumulated directly in the block-diag [128,4]
                # layout: rows 0:64 <- even heads, rows 64:128 <- odd heads
                for half in range(2):
                    sel = bass.AP(
                        tensor=wtt.tensor, offset=wtt.offset + half, ap=[wtt.ap[0], [2, 4]]
                    )
                    nc.tensor.matmul(
                        axs_ps[64 * half : 64 * (half + 1), FC * H : FC * H + 4],
                        ones64[:],
                        sel,
                        start=(t == 0),
                        stop=(t == NT - 1),
                        skip_group_check=True,
                        tile_position=(0, 64 * half),
                    )

        # ---- software-pipelined emission (per-engine queues are in-order)
        xt0 = emit_transposes(0)
        xt1 = emit_transposes(1)
        emit_copy(0, xt0)
        sc0 = emit_scores(0)
        xt2 = emit_transposes(2)
        emit_copy(1, xt1)
        emit_exp(0, sc0)
        sc1 = emit_scores(1)
        emit_attn(0)
        xt3 = emit_transposes(3)
        emit_copy(2, xt2)
        emit_exp(1, sc1)
        sc2 = emit_scores(2)
        emit_attn(1)
        emit_copy(3, xt3)
        emit_exp(2, sc2)
        sc3 = emit_scores(3)
        emit_attn(2)
        emit_exp(3, sc3)
        emit_attn(3)

        # ---- tail: recip, axc->bf16, per-head Wv matmuls straight into the
        #      extracted [128,4] layout, normalize, Wo matmuls, bias, store
        nc.vector.tensor_copy(axT_sb[:], axs_ps[:, 0 : FC * H])
        nc.vector.reciprocal(bd_sb[:], axs_ps[:, FC * H : FC * H + 4])

        afT_ps = warmp.tile([128, 4], f32, tag="warm", name="afT")
        for c in range(4):
            for half in range(2):
                head = 2 * c + half
                for fc in range(FC):
                    nc.tensor.matmul(
                        afT_ps[64 * half : 64 * (half + 1), c : c + 1],
                        wv_v(fc, c, half),
                        axT_sb[:, fc * H + head : fc * H + head + 1],
                        start=(fc == 0),
                        stop=(fc == FC - 1),
                        skip_group_check=True,
                        tile_position=(0, 64 * half),
                    )
        nc.vector.tensor_mul(ac_sb[:], afT_ps[:], bd_sb[:])

        o_ps = warmp.tile([128, FC], f32, tag="warm", name="o")
        for mc in range(FC):
            for c in range(4):
                nc.tensor.matmul(
                    o_ps[:, mc : mc + 1],
                    wo_v(c, mc),
                    ac_sb[:, c : c + 1],
                    start=(c == 0),
                    stop=(c == 3),
                    skip_group_check=True,
                )
        nc.vector.tensor_add(o_sb[:], o_ps[:], bo_v)

        if USE_KV_STORE:
            # prep emitted AFTER the o_sb writer so the deferred RAW edge
            # lands on the trigger (the prep itself runs early on the idle
            # Pool engine — it has no sync deps of its own)
            kv_sem = nc.alloc_semaphore("kv_done")
            nc.gpsimd.kv_writeback(
                out_ap=out,
                in_ap=o_sb[:].rearrange("p (a b c) -> p a b c", b=1, c=1),
                ctx_idxs_ap=idx_sb[:],
                prepare_only=True,
                sem=kv_sem,
            )
            nc.gpsimd.trigger_dma(count=None)
            nc.gpsimd.wait_ge(kv_sem, 16)
        else:
            nc.sync.dma_start(out=out, in_=o_sb[:])

    if USE_KV_STORE:
        # Tile's pass 1 gives the SWDGE prep a DMASW lane tick, so the exit
        # drain waits DMASW0 >= 16 — but with a custom sem= the increments
        # went to kv_done instead.  Append the DMASW bump to the prep's
        # updates (fires at prep completion; program end is still gated on
        # the real DMA via wait_ge(kv_done)).
        insts = [i for blk in nc.m.functions[0].blocks for i in blk.instructions]
        dmasw = None
        for i in insts:
            si = i.sync_info
            if not si:
                continue
            for w in si.on_wait:
                if (w.ant_name or "").startswith("DMASW"):
                    dmasw = w
                    break
            if dmasw:
                break
        assert dmasw is not None, "no DMASW drain wait found"
        prep_idx = next(
            k for k, i in enumerate(insts) if type(i).__name__ == "InstKVWritebackAnt"
        )
        prep = insts[prep_idx]
        upd = mybir.SyncUpdate(
            sync_type="semaphore",
            id=dmasw.id,
            ant_name=dmasw.ant_name,
            update_mode="sem-add-imm",
            update_value=dmasw.wait_value,
            update_reg=None,
        )
        si = prep.sync_info
        si.on_update = list(si.on_update) + [upd]
        # The kv prep's o_sb RAW edge is NOT in the rust swdge deferred-deps
        # table, so tile emitted a Pool EventSemaphore (wait DVE tick of the
        # o_sb writer) right before the prep — putting the ~1us descriptor
        # gen on the critical path.  Desc-gen reads no tensor data, so move
        # that wait onto the trigger instead (which is when the DMA actually
        # reads o_sb).
        trig = next(i for i in insts if type(i).__name__ == "InstTriggerDma")
        moved = []
        psi = prep.sync_info
        keep = []
        for w in psi.on_wait:
            (keep if w.ant_name.startswith("Pool") else moved).append(w)
        psi.on_wait = keep
        for k in range(prep_idx - 1, max(prep_idx - 4, -1), -1):
            ii = insts[k]
            if type(ii).__name__ == "InstEventSemaphore" and ii.sync_info:
                wsi = ii.sync_info
                ev_keep = []
                for w in wsi.on_wait:
                    (ev_keep if w.ant_name.startswith("Pool") else moved).append(w)
                wsi.on_wait = ev_keep
                break
        assert moved, "no data wait found to defer from kv prep to trigger"
        tsi = trig.sync_info
        tsi.on_wait = list(tsi.on_wait) + moved

    nc.compile()
    return nc


def get_nc():
    if "nc" not in _cache:
        _cache["nc"] = _build()
    return _cache["nc"]


def host_prep(inputs: dict) -> list[dict]:
    """Per-core input maps: bf16 x chunks (chunk0 carries folded M and bo)
    plus a shared packed bf16 Wv|Wo tensor."""
    import ml_dtypes

    bf = ml_dtypes.bfloat16
    xs = np.asarray(inputs["x"], dtype=np.float32)
    Wq = np.asarray(inputs["Wq"], dtype=np.float32)
    Wk = np.asarray(inputs["Wk"], dtype=np.float32)
    Wv = np.asarray(inputs["Wv"], dtype=np.float32)
    Wo = np.asarray(inputs["Wo"], dtype=np.float32)
    bo = np.asarray(inputs["bo"], dtype=np.float32)

    w_pack = np.concatenate(
        [
            Wv.reshape(FC, 128, PROJ).transpose(1, 0, 2).reshape(128, FC * PROJ),
            Wo.reshape(4, 128, F).transpose(1, 0, 2).reshape(128, 4 * F),
        ],
        axis=1,
    ).astype(bf)
    bo_pack = np.ascontiguousarray(bo.reshape(FC, 128).T)

    in_maps = []
    for b in range(B):
        q_row = xs[b, -1] @ Wq                                   # [512]
        M = (Wk * q_row[None, :]).reshape(F, H, D).sum(-1)       # [256, 8]
        m_pack = M.reshape(FC, 128, H).transpose(1, 0, 2).reshape(128, FC * H)
        flat = xs[b].reshape(NT, 128, F).transpose(1, 0, 2).reshape(128, NT * F)
        m = {"W": w_pack}
        for g, ct in enumerate(CHUNKS):
            lo = _CH_OFF[g] * F
            part = flat[:, lo : lo + ct * F]
            if g == 0:
                part = np.concatenate([part, m_pack, bo_pack], axis=1)
            m[f"x{g}"] = np.ascontiguousarray(part.astype(bf))
        in_maps.append(m)
    return in_maps


def run_hw(inputs: dict) -> np.ndarray:
    nc = get_nc()
    res = run_bass_kernel_spmd(nc, host_prep(inputs), list(range(B)))
    outs = []
    for b in range(B):
        o = np.asarray(res.results[b]["out"], dtype=np.float32).reshape(128, FC)
        outs.append(o.T.reshape(F))
    return np.stack(outs)


def kernel(**inputs) -> np.ndarray:
    return run_hw(inputs)


# revision 20
# speedup vs baseline: 1.1515x; 1.1013x over previous
"""Trainium2 Bass kernel: causal MHSA, last-position output (bf16 datapath).

The reference returns only out[:, -1, :]; with the causal mask the last query
row attends to everything, so per batch element the whole MHSA collapses to
tiny GEMVs.  q_row and M = Wk-contracted-with-q fold on the host (as in the
fp32 baseline); everything that streams through the device is cast to bf16 on
the host, which halves HBM traffic and runs the PE at 1 cyc/row instead of 4.

Sharding: pure data parallel over batch, core b <- batch b, no collectives.

Device-side structure per core:
  - 5 HWDGE DMAs total: x in 4 chunks (chunk 0 also carries the folded M and
    bo packed into extra columns) and one packed Wv|Wo tensor.  Fewer, larger
    DMAs matter because descriptor-gen serializes at ~650ns per DMA on the
    shared HWDGE device.
  - per chunk: PE transposes x tiles (bf16, 1cyc/row) -> PSUM, copy back to
    SBUF (DVE), scores matmuls (xT stationary, M moving, N=8), ACT exp into
    bf16 wt, attention matmuls (x stationary, wt moving, N=8) accumulating
    axc in PSUM, and softmax sums accumulated in the SAME PSUM tile in the
    block-diag [128,4] layout via 64-row tile_position matmuls (even/odd
    head columns), which makes the final normalize a plain elementwise mul.
  - tail: reciprocal of the sums block, axc -> bf16, per-head 64-row Wv
    matmuls directly into the extracted [128,4] layout (tile_position),
    normalize, Wo matmuls, bias add, store via SWDGE prepare/trigger
    (descriptor-gen off the critical path; saves ~1.2us vs a plain HWDGE
    store issued after the data is ready).
"""

import numpy as np
from contextlib import ExitStack

import concourse.bass as bass
import concourse.tile as tile
from concourse import bacc, mybir
from concourse.bass_utils import run_bass_kernel_spmd
from concourse.masks import make_identity

B, S, F, PROJ, H, D = 8, 2048, 256, 512, 8, 64
NT = S // 128         # 16 s-tiles
FC = F // 128         # 2 f-chunks
import os as _os

CHUNKS = tuple(int(v) for v in _os.environ.get("K_CHUNKS", "5,5,4,2").split(","))
NCH = len(CHUNKS)
AUX = 18              # extra cols on chunk0: M (16) + bo (2)
f32 = mybir.dt.float32
bf16 = mybir.dt.bfloat16
i32 = mybir.dt.int32
EXP = mybir.ActivationFunctionType.Exp
COPY = mybir.ActivationFunctionType.Copy

N_WARM = int(_os.environ.get("K_WARM", "20"))
SUMS_MODE = _os.environ.get("K_SUMS", "bd")  # "bd" (tile_position trick) | "old"
USE_KV_STORE = _os.environ.get("K_KV", "1") == "1"
COPY_ENG = tuple(_os.environ.get("K_COPY", "dve,dve,dve,dve").split(","))
W_SPLIT = _os.environ.get("K_WSPLIT", "1") == "1"

_cache = {}

_CH_OFF = [sum(CHUNKS[:g]) for g in range(NCH)]  # first tile of each chunk


def _build():
    nc = bacc.Bacc("TRN2", target_bir_lowering=False, debug=False, num_devices=B)
    x_dram = []
    for g, ct in enumerate(CHUNKS):
        cols = ct * F + (AUX if g == 0 else 0)
        x_dram.append(nc.dram_tensor(f"x{g}", [128, cols], bf16, kind="ExternalInput").ap())
    w_dram = nc.dram_tensor("W", [128, 2048], bf16, kind="ExternalInput").ap()
    if USE_KV_STORE:
        out = nc.dram_tensor("out", [1, 128, FC, 1], f32, kind="ExternalOutput").ap()
    else:
        out = nc.dram_tensor("out", [128, FC], f32, kind="ExternalOutput").ap()
    dbg = None
    if _os.environ.get("K_DEBUG") == "1":
        dbg = nc.dram_tensor("dbg", [128, 24 + NT * H], f32, kind="ExternalOutput").ap()

    with tile.TileContext(nc) as tc, ExitStack() as ctx:
        P = ctx.enter_context(tc.tile_pool(name="persist", bufs=1))
        xtp = ctx.enter_context(tc.tile_pool(name="xtp", bufs=2, space="PSUM"))
        sct = ctx.enter_context(tc.tile_pool(name="sct", bufs=2, space="PSUM"))
        pers = ctx.enter_context(tc.tile_pool(name="pers", bufs=1, space="PSUM"))
        warmp = ctx.enter_context(tc.tile_pool(name="warmp", bufs=1, space="PSUM"))

        ident = P.tile([128, 128], bf16)
        ones64 = P.tile([128, 64], bf16)
        scratch = P.tile([128, 128], bf16)
        x_sb = [P.tile([128, ct * F + (AUX if g == 0 else 0)], bf16, name=f"x_sb{g}")
                for g, ct in enumerate(CHUNKS)]
        xT_sb = P.tile([128, FC, S], bf16)
        wt_sb = P.tile([128, NT * H], bf16)
        axT_sb = P.tile([128, FC * H], bf16)
        bd_sb = P.tile([128, 4], f32)
        ac_sb = P.tile([128, 4], bf16)
        o_sb = P.tile([128, FC], f32)
        w_sb = P.tile([128, 2048], bf16)
        if USE_KV_STORE:
            idx_sb = P.tile([128, 1], i32)

        def xv(t, c):
            """x tile t, f-chunk c as a [128,128] SBUF view ([s-part, f])."""
            g = 0
            while g + 1 < NCH and t >= _CH_OFF[g + 1]:
                g += 1
            j = t - _CH_OFF[g]
            base = j * F + c * 128
            return x_sb[g][:, base : base + 128]

        def mv(c):
            return x_sb[0][:, CHUNKS[0] * F + c * H : CHUNKS[0] * F + (c + 1) * H]

        bo_v = x_sb[0][:, CHUNKS[0] * F + 2 * H : CHUNKS[0] * F + 2 * H + FC]

        # ---- init (gpsimd builds constants; DVE memsets warm-up scratch)
        nc.vector.memset(scratch[:], 1.0)
        make_identity(nc, ident[:])
        nc.gpsimd.memset(ones64[:], 1.0)
        if USE_KV_STORE:
            nc.gpsimd.memset(idx_sb[:], 0)

        # ---- DMAs (SP/HWDGE): x chunks then weights
        for g in range(NCH):
            nc.sync.dma_start(out=x_sb[g][:], in_=x_dram[g])
        if W_SPLIT:
            nc.sync.dma_start(out=w_sb[:, 0:1024], in_=w_dram[:, 0:1024])
            nc.sync.dma_start(out=w_sb[:, 1024:2048], in_=w_dram[:, 1024:2048])
        else:
            nc.sync.dma_start(out=w_sb[:], in_=w_dram)

        def wv_v(fc, c, half):
            return w_sb[:, fc * 512 + c * 128 + 64 * half : fc * 512 + c * 128 + 64 * half + 64]

        def wo_v(c, mc):
            return w_sb[:, 1024 + c * 256 + mc * 128 : 1024 + c * 256 + (mc + 1) * 128]

        # ---- PE warm-up (p-state ramp; content is irrelevant)
        warm_ps = warmp.tile([128, 128], bf16, tag="warm")
        for _ in range(N_WARM):
            nc.tensor.transpose(warm_ps[:], scratch[:], scratch[:])

        # persistent PSUM accumulators.  axc and sums live in SEPARATE PSUM
        # tiles (banks): on hardware a start=True matmul resets the full
        # row-range of the bank, not just its output columns, so the 64-row
        # sums matmuls would wipe axc's tile-0 contribution if co-located.
        axs_ps = pers.tile([128, FC * H], f32, tag="axs")
        sums_ps = pers.tile([128, 4], f32, tag="sums")

        def emit_transposes(g):
            xt_ps = xtp.tile([128, FC, CHUNKS[g] * 128], bf16, tag="xt", name=f"xt{g}")
            for j in range(CHUNKS[g]):
                t = _CH_OFF[g] + j
                for c in range(FC):
                    nc.tensor.transpose(
                        xt_ps[:, c, j * 128 : (j + 1) * 128], xv(t, c), ident[:]
                    )
            return xt_ps

        def emit_copy(g, xt_ps):
            lo = _CH_OFF[g] * 128
            dst = xT_sb[:, :, lo : lo + CHUNKS[g] * 128]
            eng = COPY_ENG[g]
            if eng == "dve":
                nc.vector.tensor_copy(dst, xt_ps[:])
            elif eng == "pool":
                nc.gpsimd.tensor_copy(dst, xt_ps[:])
            elif eng == "act":
                nc.scalar.activation(out=dst, in_=xt_ps[:], func=COPY)
            else:
                raise ValueError(eng)

        def emit_scores(g):
            sct_ps = sct.tile([128, CHUNKS[g] * H], f32, tag="sc", name=f"sc{g}")
            for j in range(CHUNKS[g]):
                t = _CH_OFF[g] + j
                for c in range(FC):
                    nc.tensor.matmul(
                        sct_ps[:, j * H : (j + 1) * H],
                        xT_sb[:, c, t * 128 : (t + 1) * 128],
                        mv(c),
                        start=(c == 0),
                        stop=(c == FC - 1),
                    )
            return sct_ps

        def emit_exp(g, sct_ps):
            nc.scalar.activation(
                out=wt_sb[:, _CH_OFF[g] * H : (_CH_OFF[g] + CHUNKS[g]) * H],
                in_=sct_ps[:],
                func=EXP,
                scale=0.125,
            )

        def emit_attn(g):
            for j in range(CHUNKS[g]):
                t = _CH_OFF[g] + j
                wtt = wt_sb[:, t * H : (t + 1) * H]
                for c in range(FC):
                    nc.tensor.matmul(
                        axs_ps[:, c * H : (c + 1) * H],
                        xv(t, c),
                        wtt,
                        start=(t == 0),
                        stop=(t == NT - 1),
                        skip_group_check=True,
                    )
                # softmax sums accumulated directly in the block-diag [128,4]
                # layout: rows 0:64 <- even heads, rows 64:128 <- odd heads
                for half in range(2):
                    sel = bass.AP(
                        tensor=wtt.tensor, offset=wtt.offset + half, ap=[wtt.ap[0], [2, 4]]
                    )
                    nc.tensor.matmul(
                        sums_ps[64 * half : 64 * (half + 1), :],
                        ones64[:],
                        sel,
                        start=(t == 0),
                        stop=(t == NT - 1),
                        skip_group_check=True,
                        tile_position=(0, 64 * half),
                    )

        # ---- software-pipelined emission (per-engine queues are in-order):
        #      transposes run two chunks ahead of the copy/scores/exp/attn
        #      stream so the PE never stalls on the current chunk's copy
        xts = {0: emit_transposes(0)}
        if NCH > 1:
            xts[1] = emit_transposes(1)
        for g in range(NCH):
            emit_copy(g, xts.pop(g))
            sc_g = emit_scores(g)
            if g + 2 < NCH:
                xts[g + 2] = emit_transposes(g + 2)
            emit_exp(g, sc_g)
            emit_attn(g)

        # ---- tail: recip, axc->bf16, per-head Wv matmuls straight into the
        #      extracted [128,4] layout, normalize, Wo matmuls, bias, store
        nc.vector.tensor_copy(axT_sb[:], axs_ps[:])
        nc.vector.reciprocal(bd_sb[:], sums_ps[:])
        if dbg is not None:
            dbg_sb = P.tile([128, 24 + NT * H], f32, name="dbg_sb")
            nc.vector.tensor_copy(dbg_sb[:, 0:16], axs_ps[:])
            nc.vector.tensor_copy(dbg_sb[:, 16:20], sums_ps[:])
            nc.vector.tensor_copy(dbg_sb[:, 20:24], bd_sb[:])
            nc.vector.tensor_copy(dbg_sb[:, 24 : 24 + NT * H], wt_sb[:])
            nc.sync.dma_start(out=dbg, in_=dbg_sb[:])

        afT_ps = warmp.tile([128, 4], f32, tag="warm", name="afT")
        for c in range(4):
            for half in range(2):
                head = 2 * c + half
                for fc in range(FC):
                    nc.tensor.matmul(
                        afT_ps[64 * half : 64 * (half + 1), c : c + 1],
                        wv_v(fc, c, half),
                        axT_sb[:, fc * H + head : fc * H + head + 1],
                        start=(fc == 0),
                        stop=(fc == FC - 1),
                        skip_group_check=True,
                        tile_position=(0, 64 * half),
                    )
        nc.vector.tensor_mul(ac_sb[:], afT_ps[:], bd_sb[:])

        o_ps = warmp.tile([128, FC], f32, tag="warm", name="o")
        for mc in range(FC):
            for c in range(4):
                nc.tensor.matmul(
                    o_ps[:, mc : mc + 1],
                    wo_v(c, mc),
                    ac_sb[:, c : c + 1],
                    start=(c == 0),
                    stop=(c == 3),
                    skip_group_check=True,
                )
        nc.vector.tensor_add(o_sb[:], o_ps[:], bo_v)

        if USE_KV_STORE:
            # prep emitted AFTER the o_sb writer so the deferred RAW edge
            # lands on the trigger (the prep itself runs early on the idle
            # Pool engine — it has no sync deps of its own)
            kv_sem = nc.alloc_semaphore("kv_done")
            nc.gpsimd.kv_writeback(
                out_ap=out,
                in_ap=o_sb[:].rearrange("p (a b c) -> p a b c", b=1, c=1),
                ctx_idxs_ap=idx_sb[:],
                prepare_only=True,
                sem=kv_sem,
            )
            nc.gpsimd.trigger_dma(count=None)
            nc.gpsimd.wait_ge(kv_sem, 16)
        else:
            nc.sync.dma_start(out=out, in_=o_sb[:])

    if USE_KV_STORE:
        # Tile's pass 1 gives the SWDGE prep a DMASW lane tick, so the exit
        # drain waits DMASW0 >= 16 — but with a custom sem= the increments
        # went to kv_done instead.  Append the DMASW bump to the prep's
        # updates (fires at prep completion; program end is still gated on
        # the real DMA via wait_ge(kv_done)).
        insts = [i for blk in nc.m.functions[0].blocks for i in blk.instructions]
        dmasw = None
        for i in insts:
            si = i.sync_info
            if not si:
                continue
            for w in si.on_wait:
                if (w.ant_name or "").startswith("DMASW"):
                    dmasw = w
                    break
            if dmasw:
                break
        assert dmasw is not None, "no DMASW drain wait found"
        prep_idx = next(
            k for k, i in enumerate(insts) if type(i).__name__ == "InstKVWritebackAnt"
        )
        prep = insts[prep_idx]
        upd = mybir.SyncUpdate(
            sync_type="semaphore",
            id=dmasw.id,
            ant_name=dmasw.ant_name,
            update_mode="sem-add-imm",
            update_value=dmasw.wait_value,
            update_reg=None,
        )
        si = prep.sync_info
        si.on_update = list(si.on_update) + [upd]
        # The kv prep's o_sb RAW edge is NOT in the rust swdge deferred-deps
        # table, so tile emitted a Pool EventSemaphore (wait DVE tick of the
        # o_sb writer) right before the prep — putting the ~1us descriptor
        # gen on the critical path.  Desc-gen reads no tensor data, so move
        # that wait onto the trigger instead (which is when the DMA actually
        # reads o_sb).
        trig = next(i for i in insts if type(i).__name__ == "InstTriggerDma")
        moved = []
        psi = prep.sync_info
        keep = []
        for w in psi.on_wait:
            (keep if w.ant_name.startswith("Pool") else moved).append(w)
        psi.on_wait = keep
        for k in range(prep_idx - 1, max(prep_idx - 4, -1), -1):
            ii = insts[k]
            if type(ii).__name__ == "InstEventSemaphore" and ii.sync_info:
                wsi = ii.sync_info
                ev_keep = []
                for w in wsi.on_wait:
                    (ev_keep if w.ant_name.startswith("Pool") else moved).append(w)
                wsi.on_wait = ev_keep
                break
        assert moved, "no data wait found to defer from kv prep to trigger"
        tsi = trig.sync_info
        tsi.on_wait = list(tsi.on_wait) + moved

    nc.compile()
    return nc


def get_nc():
    if "nc" not in _cache:
        _cache["nc"] = _build()
    return _cache["nc"]


def host_prep(inputs: dict) -> list[dict]:
    """Per-core input maps: bf16 x chunks (chunk0 carries folded M and bo)
    plus a shared packed bf16 Wv|Wo tensor."""
    import ml_dtypes

    bf = ml_dtypes.bfloat16
    xs = np.asarray(inputs["x"], dtype=np.float32)
    Wq = np.asarray(inputs["Wq"], dtype=np.float32)
    Wk = np.asarray(inputs["Wk"], dtype=np.float32)
    Wv = np.asarray(inputs["Wv"], dtype=np.float32)
    Wo = np.asarray(inputs["Wo"], dtype=np.float32)
    bo = np.asarray(inputs["bo"], dtype=np.float32)

    w_pack = np.concatenate(
        [
            Wv.reshape(FC, 128, PROJ).transpose(1, 0, 2).reshape(128, FC * PROJ),
            Wo.reshape(4, 128, F).transpose(1, 0, 2).reshape(128, 4 * F),
        ],
        axis=1,
    ).astype(bf)
    bo_pack = np.ascontiguousarray(bo.reshape(FC, 128).T)

    in_maps = []
    for b in range(B):
        q_row = xs[b, -1] @ Wq                                   # [512]
        M = (Wk * q_row[None, :]).reshape(F, H, D).sum(-1)       # [256, 8]
        m_pack = M.reshape(FC, 128, H).transpose(1, 0, 2).reshape(128, FC * H)
        flat = xs[b].reshape(NT, 128, F).transpose(1, 0, 2).reshape(128, NT * F)
        m = {"W": w_pack}
        for g, ct in enumerate(CHUNKS):
            lo = _CH_OFF[g] * F
            part = flat[:, lo : lo + ct * F]
            if g == 0:
                part = np.concatenate([part, m_pack, bo_pack], axis=1)
            m[f"x{g}"] = np.ascontiguousarray(part.astype(bf))
        in_maps.append(m)
    return in_maps


def run_hw_raw(inputs: dict):
    nc = get_nc()
    return run_bass_kernel_spmd(nc, host_prep(inputs), list(range(B)))


def run_hw(inputs: dict) -> np.ndarray:
    nc = get_nc()
    res = run_bass_kernel_spmd(nc, host_prep(inputs), list(range(B)))
    outs = []
    for b in range(B):
        o = np.asarray(res.results[b]["out"], dtype=np.float32).reshape(128, FC)
        outs.append(o.T.reshape(F))
    return np.stack(outs)


def kernel(**inputs) -> np.ndarray:
    return run_hw(inputs)
